# revision 48
# baseline (speedup 1.0000x reference)
# Multi-head causal attention (B=4, S=2048, D=1024, H=16, Dh=64) on 8 trn2 cores.
#
# Sharding: tensor-parallel over heads — core c owns heads (2c, 2c+1) for all
# batches. Each core projects Q/K/V for its 2 heads, runs causal attention, and
# computes a partial output projection against its 128 rows of w_o. The host
# sums the 8 partial outputs (the "all-reduce").
#
# Layouts (chosen so no transposes are needed on the attention path):
#   embedT   [B, 128, 8, S]  fp8e4  (q/k path; DoubleRow moving operand)
#   embedT16 [B, 128, 8, S]  bf16   (v path — fp8 V fails the accuracy gate:
#            h is a ~1000-key weighted average, so v's quantization noise
#            doesn't average down relative to h's small magnitude)
#   wq2/wk2  [128, 8, 128] fp8e4, prescaled x32 (clears e4m3 subnormals);
#            the 1/(32*32) comes back out inside the exp scale immediate
#   wv2      [128, 8, 128] bf16;  wo2 [128, 1024] bf16
#   Scores are computed transposed: sT[k, q] = sum_dh kT[dh,k] qT[dh,q]. The
#   Q/K projections run as fp8 DoubleRow matmuls (256-deep contraction, two
#   d-chunks per pass — half the PE passes of bf16). The softmax denominator
#   comes from a ones-column appended to V (the PV matmul accumulates both
#   the PV product and the exp-sum), and exp'd scores feed the PV matmul
#   directly as the moving operand.
#
# Engine assignment: PE is the bottleneck; causal masking runs as
# affine_select on gpsimd; ACT does exp (+ tail copies); DVE does PSUM evac;
# the normalize multiplies ride gpsimd (SBUF-only operands). Warm-up matmuls
# on garbage SBUF run during the initial DMA wait and inside phase_a's gaps
# so the PE's HAM clock gate reaches 8/8 (2.4 GHz) before the real stream.
#
# Softmax normalization: den row -> [128,8] reshape (SBUF->SBUF DMA) ->
# reciprocal -> DRAM -> [[0,64],[1,1024]] broadcast read. All stages after
# the reshape are DEFERRED (pend callbacks with chunk-count deadlines) so no
# queue head-blocks on an in-flight DMA. The final q-block of the final
# batch instead folds normalization into a per-head split of the output
# projection (row-tiled 64-contraction pairs) scaled by per-partition
# reciprocals — no broadcast DMA on the kernel tail's critical path.
import numpy as np
import ml_dtypes

B, S, D, H, Dh = 4, 2048, 1024, 16, 64
NCORES = 8
HPC = H // NCORES          # heads per core = 2
DC = D // 128              # d chunks = 8
NQB = S // 512             # q blocks = 4
NKB = S // 128             # k chunks = 16
NST = S // 128             # s tiles = 16
WSCALE = 32.0              # fp8 q/k weight prescale (clears e4m3 subnormals)
# 1/sqrt(Dh) folded into exp, divided by the q*k weight prescales
ESCALE = 1.0 / (8.0 * WSCALE * WSCALE)

_cache = {}


def _build_nc():
    import concourse.bass as bass
    import concourse.mybir as mybir
    import concourse.tile as tile
    from concourse import bacc

    bf16 = mybir.dt.bfloat16
    f32 = mybir.dt.float32
    f8 = mybir.dt.float8e4
    DRM = mybir.MatmulPerfMode.DoubleRow
    EXP = mybir.ActivationFunctionType.Exp
    GE = mybir.AluOpType.is_ge
    MUL = mybir.AluOpType.mult
    ADD = mybir.AluOpType.add

    nc = bacc.Bacc("TRN2", target_bir_lowering=False, debug=False,
                   num_devices=NCORES)

    embedT = nc.dram_tensor("embedT", [B, 128, DC, S], f8, kind="ExternalInput")
    embedT16 = nc.dram_tensor("embedT16", [B, 128, DC, S], bf16,
                              kind="ExternalInput")
    wq2 = nc.dram_tensor("wq2", [128, DC, 128], f8, kind="ExternalInput")
    wk2 = nc.dram_tensor("wk2", [128, DC, 128], f8, kind="ExternalInput")
    wv2 = nc.dram_tensor("wv2", [128, DC, 128], bf16, kind="ExternalInput")
    wo2 = nc.dram_tensor("wo2", [128, D], bf16, kind="ExternalInput")
    identin = nc.dram_tensor("identin", [128, 128], bf16, kind="ExternalInput")
    outp = nc.dram_tensor("outp", [B, S, D], bf16, kind="ExternalOutput")

    with tile.TileContext(nc) as tc:
        with (
            tc.tile_pool(name="const", bufs=1) as const,
            tc.tile_pool(name="etp", bufs=2) as etp,
            tc.tile_pool(name="etvp", bufs=2) as etvp,
            tc.tile_pool(name="qkp", bufs=2) as qkp,
            tc.tile_pool(name="vxp", bufs=2) as vxp,
            tc.tile_pool(name="hdp", bufs=2) as hdp,
            tc.tile_pool(name="expp", bufs=4) as expp,
            tc.tile_pool(name="denp", bufs=4) as denp,
            tc.tile_pool(name="outs", bufs=3) as outs,
            tc.tile_pool(name="tmpp", bufs=2) as tmpp,
            tc.tile_pool(name="pscore", bufs=2, space="PSUM") as pscore,
            tc.tile_pool(name="ppv", bufs=1, space="PSUM") as ppv,
            tc.tile_pool(name="pproj", bufs=2, space="PSUM") as pproj,
            tc.tile_pool(name="drp", bufs=8, space="DRAM") as drp,
        ):
            # HAM warm-up: tiny matmuls on zeroed SBUF, issued before
            # anything else so the PE is busy while the first weight and
            # embedding DMAs are in flight; by the time real matmuls start
            # the clock gate is at 8/8 instead of 4/8.
            junk = const.tile([128, 128], bf16, tag="junk")
            nc.vector.memset(junk[:], 0.0)
            wps = pscore.tile([128, 1024], f32, tag="score")
            for _ in range(96):
                nc.tensor.matmul(wps[:, 0:128], junk[:], junk[:])

            wq_sb = const.tile([128, DC, 128], f8, tag="wq")
            wk_sb = const.tile([128, DC, 128], f8, tag="wk")
            wv_sb = const.tile([128, DC, 128], bf16, tag="wv")
            wo_sb = const.tile([128, D], bf16, tag="wo")
            ident_sb = const.tile([128, 128], bf16, tag="ident")

            def load_et(b, prologue=False):
                """fp8 q/k-path chunks (gpsimd queue) + bf16 v-path chunks
                (sync queue; 4KB per-partition lines keep the DMA efficient).
                In the prologue the first fp8 dc pair and the weights lead
                both queues so phase_a starts as early as possible."""
                et = etp.tile([128, DC, S], f8, tag="et")
                etv = etvp.tile([128, DC, S], bf16, tag="etv")

                def chunk(dc):  # dc-pair: 4KB contiguous lines
                    nc.gpsimd.dma_start(out=et[:, dc:dc + 2, :],
                                        in_=embedT[b, :, dc:dc + 2, :])

                def chunkv(dc):  # dc-pair: 8KB contiguous lines
                    nc.sync.dma_start(out=etv[:, dc:dc + 2, :],
                                      in_=embedT16[b, :, dc:dc + 2, :])
                if prologue:
                    chunk(0)
                    nc.gpsimd.dma_start(out=wq_sb[:], in_=wq2[:])
                    nc.gpsimd.dma_start(out=wk_sb[:], in_=wk2[:])
                    nc.sync.dma_start(out=wv_sb[:], in_=wv2[:])
                    chunkv(0)
                    nc.sync.dma_start(out=ident_sb[:], in_=identin[:])
                    for dc in range(2, DC, 2):
                        chunk(dc)
                    nc.sync.dma_start(out=wo_sb[:], in_=wo2[:])
                    chunkv(2)
                    chunkv(4)
                    # balance the prologue queues: the last v-path pair rides
                    # gpsimd (sync otherwise carries ~2x the bytes)
                    nc.gpsimd.dma_start(out=etv[:, 6:8, :],
                                        in_=embedT16[b, :, 6:8, :])
                else:
                    for dc in range(0, DC, 2):
                        chunk(dc)
                    for dc in range(0, DC, 2):
                        chunkv(dc)
                return et, etv

            def make_proj_units(ets, b, prologue=False):
                """Units producing qT2, kT2, vext0/1 for one batch. Q/K are
                fp8 DoubleRow from `et`; V is bf16 from `etv`."""
                et, etv = ets
                qT2 = qkp.tile([128, S], bf16, tag="qT2")
                kT2 = qkp.tile([128, S], bf16, tag="kT2")
                vT2 = qkp.tile([128, S], bf16, tag="vT2")
                vext0 = vxp.tile([128, NKB, 65], bf16, tag="vext0")
                vext1 = vxp.tile([128, NKB, 65], bf16, tag="vext1")
                units = []

                def ones_u():
                    nc.gpsimd.memset(vext0[:, :, 64:65], 1.0)
                    nc.gpsimd.memset(vext1[:, :, 64:65], 1.0)
                units.append(ones_u)
                if prologue:
                    # dc-major over the first q/k blocks so the PE starts as
                    # soon as the first embedding chunk pair lands; warm-up
                    # matmuls fill the DMA-wait gaps
                    def phase_a():
                        ps_q = pproj.tile([128, 512], f32, tag="proj")
                        ps_k = pproj.tile([128, 512], f32, tag="proj")
                        for dc in range(0, DC, 2):
                            nc.tensor.matmul(ps_q[:], wq_sb[:, dc:dc + 2, :],
                                             et[:, dc:dc + 2, 0:512],
                                             start=(dc == 0),
                                             stop=(dc == DC - 2),
                                             perf_mode=DRM)
                            nc.tensor.matmul(ps_k[:], wk_sb[:, dc:dc + 2, :],
                                             et[:, dc:dc + 2, 0:512],
                                             start=(dc == 0),
                                             stop=(dc == DC - 2),
                                             perf_mode=DRM)
                            for _ in range(4):
                                nc.tensor.matmul(wps[:, 0:128], junk[:],
                                                 junk[:])
                        nc.vector.tensor_copy(out=qT2[:, 0:512], in_=ps_q[:])
                        nc.vector.tensor_copy(out=kT2[:, 0:512], in_=ps_k[:])
                    units.append(phase_a)
                for dst, w_sb in ((qT2, wq_sb), (kT2, wk_sb)):
                    for sblk in range(S // 512):
                        if prologue and sblk == 0:
                            continue
                        def proj_u(dst=dst, w_sb=w_sb, sblk=sblk):
                            ps = pproj.tile([128, 512], f32, tag="proj")
                            for dc in range(0, DC, 2):
                                nc.tensor.matmul(
                                    ps[:], w_sb[:, dc:dc + 2, :],
                                    et[:, dc:dc + 2,
                                        sblk * 512:(sblk + 1) * 512],
                                    start=(dc == 0), stop=(dc == DC - 2),
                                    perf_mode=DRM)
                            nc.vector.tensor_copy(
                                out=dst[:, sblk * 512:(sblk + 1) * 512],
                                in_=ps[:])
                        units.append(proj_u)
                # v path: bf16 from etv
                for sblk in range(S // 512):
                    def vproj_u(sblk=sblk):
                        ps = pproj.tile([128, 512], f32, tag="proj")
                        for dc in range(DC):
                            nc.tensor.matmul(
                                ps[:], wv_sb[:, dc, :],
                                etv[:, dc, sblk * 512:(sblk + 1) * 512],
                                start=(dc == 0), stop=(dc == DC - 1))
                        nc.vector.tensor_copy(
                            out=vT2[:, sblk * 512:(sblk + 1) * 512], in_=ps[:])
                    units.append(vproj_u)
                for st in range(NST):
                    def tr_u(st=st):
                        vt = pproj.tile([128, 128], bf16, tag="proj")
                        nc.tensor.transpose(
                            vt[:], vT2[:, st * 128:(st + 1) * 128], ident_sb[:])
                        nc.vector.tensor_copy(out=vext0[:, st, 0:64],
                                              in_=vt[:, 0:64])
                        nc.vector.tensor_copy(out=vext1[:, st, 0:64],
                                              in_=vt[:, 64:128])
                    units.append(tr_u)
                return units, (qT2, kT2, vext0, vext1)

            def make_outproj_units(bb, hq, sts=range(NST), act_half=False):
                """Outproj units; consecutive (even,odd) s-tiles share one
                [128, 2, 1024] output tile and a single paired 512KB store,
                halving DMA-issue count. Stores alternate gpsimd/sync."""
                units = []
                state = {}
                sts = list(sts)
                for idx, st in enumerate(sts):
                    def op_u(st=st, idx=idx):
                        hs = hq[st // 4][:, (st % 4) * 128:(st % 4 + 1) * 128]
                        po0 = pproj.tile([128, 512], f32, tag="proj")
                        nc.tensor.matmul(po0[:], hs, wo_sb[:, 0:512])
                        po1 = pproj.tile([128, 512], f32, tag="proj")
                        nc.tensor.matmul(po1[:], hs, wo_sb[:, 512:1024])
                        if idx % 2 == 0:
                            state["ob2"] = outs.tile([128, 2, 1024], bf16,
                                                     tag="ob2", name="ob2")
                        ob = state["ob2"][:, idx % 2, :]
                        if act_half:
                            # splitting one evac to the (then-idle) ACT keeps
                            # the PSUM slots recycling at PE pace
                            nc.scalar.copy(out=ob[:, 0:512], in_=po0[:])
                        else:
                            nc.vector.tensor_copy(out=ob[:, 0:512], in_=po0[:])
                        nc.vector.tensor_copy(out=ob[:, 512:1024], in_=po1[:])
                        if idx % 2 == 1:
                            emit_store(bb, st - 1, state["ob2"])
                    units.append(op_u)
                return units

            def emit_store(bb, st0, ob2):
                # paired store: rows [st0*128, st0*128+256) of outp
                d0 = outp[bb, st0 * 128:(st0 + 1) * 128, :]
                dst = bass.AP(tensor=d0.tensor, offset=d0.offset,
                              ap=[[D, 128], [128 * D, 2], [1, D]])
                q = nc.gpsimd if (st0 // 2) % 2 == 0 else nc.sync
                q.dma_start(out=dst, in_=ob2[:])

            def tail_outproj_units(bb, hu, denct):
                """Final q-block of the final batch: h is UNnormalized; the
                output projection is split per head (row-tiled 64-contraction
                pairs run concurrently on the PE) and the per-q reciprocal
                denominators are applied as per-partition scalars during the
                PSUM combine — no broadcast DMA on the tail critical path."""
                units = []
                state = {}
                for idx, st in enumerate(range(12, 16)):
                    def op_u(st=st, idx=idx):
                        sub = st % 4
                        hs0 = hu[0:64, sub * 128:(sub + 1) * 128]
                        hs1 = hu[64:128, sub * 128:(sub + 1) * 128]
                        r0 = denct[:, 8 + sub:9 + sub]
                        r1 = denct[:, 12 + sub:13 + sub]
                        ph0 = pscore.tile([128, 1024], f32, tag="score")
                        ph1 = pscore.tile([128, 1024], f32, tag="score")
                        nc.tensor.matmul(ph0[:, 0:512], hs0, wo_sb[0:64, 0:512])
                        nc.tensor.matmul(ph1[:, 0:512], hs1,
                                         wo_sb[64:128, 0:512])
                        nc.tensor.matmul(ph0[:, 512:1024], hs0,
                                         wo_sb[0:64, 512:1024])
                        nc.tensor.matmul(ph1[:, 512:1024], hs1,
                                         wo_sb[64:128, 512:1024])
                        if idx % 2 == 0:
                            state["ob2"] = outs.tile([128, 2, 1024], bf16,
                                                     tag="ob2", name="ob2")
                        ob = state["ob2"][:, idx % 2, :]
                        tmp0 = tmpp.tile([128, 512], f32, tag="tmp0",
                                         name="tmp0")
                        tmp1 = tmpp.tile([128, 512], f32, tag="tmp1",
                                         name="tmp1")
                        # ob = po_h0 * (1/den_h0[q]) + po_h1 * (1/den_h1[q]),
                        # den indexed per PARTITION (q) in this layout; the
                        # first product rides the (tail-idle) ACT so the DVE
                        # only runs the fused multiply-adds
                        nc.scalar.activation(
                            out=tmp0[:], in_=ph0[:, 0:512],
                            func=mybir.ActivationFunctionType.Copy, scale=r0)
                        nc.vector.scalar_tensor_tensor(
                            ob[:, 0:512], ph1[:, 0:512], r1, tmp0[:],
                            op0=MUL, op1=ADD)
                        nc.scalar.activation(
                            out=tmp1[:], in_=ph0[:, 512:1024],
                            func=mybir.ActivationFunctionType.Copy, scale=r0)
                        nc.vector.scalar_tensor_tensor(
                            ob[:, 512:1024], ph1[:, 512:1024], r1, tmp1[:],
                            op0=MUL, op1=ADD)
                        if idx % 2 == 1:
                            emit_store(bb, st - 1, state["ob2"])
                    units.append(op_u)
                return units

            def run_attention(b, proj_tiles, units, hq, pend, gct, last=False,
                              post=None):
                """Attention for batch b; `units` paced into the kb loop.
                `pend` carries deferred (deadline, callback) stages — the
                reciprocal, its DMAs, and the normalize multiplies — so no
                engine stream ever waits on an in-flight DMA. `gct` is a
                global chunk counter (survives across batches)."""
                qT2, kT2, vext0, vext1 = proj_tiles
                nslots = sum(4 * qb + 4 for qb in range(NQB))
                t = 0
                tail = {}
                for qb in range(NQB):
                    qs = slice(qb * 512, (qb + 1) * 512)
                    pv0 = ppv.tile([65, 512], f32, tag="pv0")
                    pv1 = ppv.tile([65, 512], f32, tag="pv1")
                    nkb = 4 * qb + 4
                    exs = [None] * nkb

                    def emit_qk(kb):
                        ps = pscore.tile([128, 1024], f32, tag="score")
                        ks = slice(kb * 128, (kb + 1) * 128)
                        diag = kb >= 4 * qb
                        # columns left of the diagonal triangle are fully
                        # masked — skip them in the QK stream too
                        w0 = (kb - 4 * qb) * 128 if diag else 0
                        qsw = slice(qb * 512 + w0, (qb + 1) * 512)
                        nc.tensor.matmul(ps[:, w0:512], kT2[0:64, ks],
                                         qT2[0:64, qsw])
                        nc.tensor.matmul(ps[:, 512 + w0:1024], kT2[64:128, ks],
                                         qT2[64:128, qsw])
                        ex = expp.tile([128, 1024], bf16, tag="ex")
                        if diag:
                            # exp covers [w0:512]; the 128-wide triangle block
                            # is then zeroed below the diagonal by an
                            # affine_select on the gpsimd engine (iota =
                            # q_local - k_local, keep iff >= 0)
                            ex3 = ex.rearrange("p (h n) -> p h n", h=2)
                            ps3 = ps.rearrange("p (h n) -> p h n", h=2)
                            nc.scalar.activation(out=ex3[:, :, w0:512],
                                                 in_=ps3[:, :, w0:512],
                                                 func=EXP, scale=ESCALE)
                            for h in (0, 1):
                                nc.gpsimd.affine_select(
                                    out=ex3[:, h, w0:w0 + 128],
                                    in_=ex3[:, h, w0:w0 + 128],
                                    pattern=[[1, 128]],
                                    compare_op=GE, fill=0.0,
                                    channel_multiplier=-1)
                        else:
                            nc.scalar.activation(out=ex[:], in_=ps[:],
                                                 func=EXP, scale=ESCALE)
                        exs[kb] = ex

                    def emit_pv(kb):
                        # diagonal chunks contribute nothing to q-columns left
                        # of the triangle block; skip them (ex is garbage
                        # there — it was never exp'd).
                        first, last_ = (kb == 0), (kb == nkb - 1)
                        w0 = (kb - 4 * qb) * 128 if kb >= 4 * qb else 0
                        ex = exs[kb]
                        nc.tensor.matmul(pv0[:, w0:512],
                                         vext0[:, kb, :],
                                         ex[:, w0:512],
                                         start=first, stop=last_)
                        nc.tensor.matmul(pv1[:, w0:512],
                                         vext1[:, kb, :],
                                         ex[:, 512 + w0:1024],
                                         start=first, stop=last_)

                    for kb in range(nkb):
                        emit_qk(kb)
                        t += 1
                        gct[0] += 1
                        # flush deferred den-chain stages whose deadlines
                        # passed; deadlines give each stage's input DMA ~3
                        # chunks to land so nothing head-blocks a queue
                        while pend and gct[0] >= pend[0][0]:
                            pend.pop(0)[1]()
                        if units:
                            u = units.pop(0)
                            if u is not None:
                                u()
                        while units and len(units) > nslots - t:
                            u = units.pop(0)
                            if u is not None:
                                u()
                        if kb > 1:
                            emit_pv(kb - 2)
                    emit_pv(nkb - 2)
                    emit_pv(nkb - 1)

                    if last and qb == NQB - 1:
                        # tail path: unnormalized h straight from PSUM (bf16),
                        # and the den row reshaped so that denct[p, 4h+sub]
                        # = den_h[sub*128 + p] — per-PARTITION reciprocals
                        # for the per-head split output projection.
                        hu = hq[qb]
                        # den rows stage through SBUF on the ACT (runs in
                        # parallel with the DVE h copies); DMA cannot read
                        # PSUM directly
                        denrow = denp.tile([1, 1024], f32, tag="denrow",
                                           name="denrow", bufs=1)
                        nc.scalar.copy(out=denrow[0:1, 0:512],
                                       in_=pv0[64:65, :])
                        nc.scalar.copy(out=denrow[0:1, 512:1024],
                                       in_=pv1[64:65, :])
                        nc.vector.tensor_copy(out=hu[0:64, :],
                                              in_=pv0[0:64, :])
                        nc.vector.tensor_copy(out=hu[64:128, :],
                                              in_=pv1[0:64, :])
                        denct = denp.tile([128, 16], f32, tag="denc")
                        # transposed reshape via a DRAM bounce (the SBUF->SBUF
                        # balancer can't split a partition-transposing AP):
                        # denct[p, 4h+j] = den_h[j*128 + p]
                        ddenr = drp.tile([1, 1024], f32, tag="dden2")
                        nc.sync.dma_start(out=ddenr[:], in_=denrow[:])
                        gap = bass.AP(tensor=ddenr.tensor, offset=ddenr.offset,
                                      ap=[[1, 128], [128, 8]])
                        nc.sync.dma_start(out=denct[:, 0:8], in_=gap)
                        nc.vector.reciprocal(out=denct[:, 8:16],
                                             in_=denct[:, 0:8])
                        tail["denct"] = denct
                        continue

                    # normalize, in deferred stages:
                    #   A (now):      pvs copy to SBUF (frees the PSUM slot)
                    #                 + den-row reshape to [128,8] via
                    #                 SBUF->SBUF DMA
                    #   B (+3 chunks): reciprocal [128,8] (its DMA input has
                    #                 landed, so the DVE never stalls), then
                    #                 recip -> DRAM -> [[0,64],[1,1024]]
                    #                 broadcast read (partition broadcast)
                    #   C (+7 chunks): normalize multiplies (gpsimd — all
                    #                 SBUF operands; keeps the DVE free for
                    #                 PSUM evacuation)
                    pvs2 = denp.tile([65, 1024], f32, tag="pvs")
                    nc.vector.tensor_copy(out=pvs2[:, 0:512], in_=pv0[:])
                    nc.vector.tensor_copy(out=pvs2[:, 512:1024], in_=pv1[:])
                    denc = denp.tile([128, 16], f32, tag="denc")
                    # SBUF->SBUF reshape: dst iterates partition-major so
                    # denc[p, j] = den_row[8p + j] in one DMA (p<64: head0)
                    nc.sync.dma_start(out=denc[:, 0:8], in_=pvs2[64:65, :])
                    dden2 = drp.tile([1, 1024], f32, tag="dden2")
                    den2 = denp.tile([64, 1024], f32, tag="den")

                    def stage_b(denc=denc, dden2=dden2, den2=den2):
                        nc.vector.reciprocal(out=denc[:, 8:16],
                                             in_=denc[:, 0:8])
                        nc.sync.dma_start(out=dden2[:], in_=denc[:, 8:16])
                        bcap = bass.AP(tensor=dden2.tensor,
                                       offset=dden2.offset,
                                       ap=[[0, 64], [1, 1024]])
                        nc.sync.dma_start(out=den2[:], in_=bcap)

                    def stage_c(pvs2=pvs2, den2=den2, ht2=hq[qb]):
                        for h in (0, 1):
                            nc.vector.tensor_mul(
                                ht2[h * 64:(h + 1) * 64, :],
                                pvs2[0:64, h * 512:(h + 1) * 512],
                                den2[:, h * 512:(h + 1) * 512])

                    db, dcm = (3, 5) if last else (3, 7)
                    pend.append((gct[0] + db, stage_b))
                    pend.append((gct[0] + dcm, stage_c))
                    # outproj units consuming this qb's h MUST be emitted
                    # after stage_c: the tile framework links a read emitted
                    # before its writer to the slot's PREVIOUS write (stale
                    # data), so emission order is a correctness requirement
                    if post and qb in post:
                        for u in post[qb]:
                            pend.append((gct[0] + dcm + 1, u))
                while units:
                    u = units.pop(0)
                    if u is not None:
                        u()
                return tail

            # prologue: batch 0 projections run standalone
            et0 = load_et(0, prologue=True)
            units0, tiles0 = make_proj_units(et0, 0, prologue=True)
            for u in units0:
                u()

            cur_tiles = tiles0
            prev_hq = None
            pend = []
            gct = [0]
            for b in range(B):
                hq = [hdp.tile([128, 512], bf16, tag=f"h{i}", name=f"hq{i}")
                      for i in range(NQB)]
                units = []
                if b > 0:
                    # st12-15 of the previous batch read hq[3] whose muls
                    # only flush a few chunks into THIS batch — pend-gate
                    # them behind that flush (emission order = correctness)
                    units += make_outproj_units(b - 1, prev_hq, range(0, 12))
                    gdl = pend[-1][0] + 1 if pend else 0
                    for u in make_outproj_units(b - 1, prev_hq,
                                                range(12, 16)):
                        pend.append((gdl, u))
                post = None
                if b + 1 < B:
                    et_n = load_et(b + 1)
                    punits, next_tiles = make_proj_units(et_n, b + 1)
                    # interleave: outproj units first (their deps are ready
                    # while et(b+1) is still streaming in), then alternate
                    k = min(8, len(units))
                    head, rest = units[:k], units[k:]
                    mixed = []
                    i = j = 0
                    while i < len(rest) or j < len(punits):
                        if j < len(punits):
                            mixed.append(punits[j]); j += 1
                        if i < len(rest):
                            mixed.append(rest[i]); i += 1
                    units = head + mixed
                else:
                    next_tiles = None
                    # last batch: its own outproj units are pend-gated so
                    # each group is emitted only after its qb's normalize
                    # muls (see run_attention)
                    post = {
                        0: make_outproj_units(b, hq, range(0, 4)),
                        1: make_outproj_units(b, hq, range(4, 8)),
                        2: make_outproj_units(b, hq, range(8, 12),
                                              act_half=True),
                    }
                tail = run_attention(b, cur_tiles, units, hq, pend, gct,
                                     last=(b == B - 1), post=post)
                cur_tiles = next_tiles
                prev_hq = hq

            for _, fn in pend:
                fn()
            for u in tail_outproj_units(B - 1, prev_hq[NQB - 1],
                                        tail["denct"]):
                u()

    nc.compile()
    return nc


def _host_prep(embed, w_q, w_k, w_v, w_o):
    bf = ml_dtypes.bfloat16
    f8 = ml_dtypes.float8_e4m3  # bit-compatible with TRN fp8e4 for |x|<=240
    embedT16 = np.ascontiguousarray(
        embed.reshape(B, S, DC, 128).transpose(0, 3, 2, 1)).astype(bf)
    embedT = embedT16.astype(f8)
    ident = np.ascontiguousarray(np.eye(128, dtype=np.float32).astype(bf))

    in_maps = []
    for c in range(NCORES):
        h0, h1 = HPC * c, HPC * c + 1
        wq_cat = np.concatenate([w_q[h0], w_q[h1]], axis=1)
        wk_cat = np.concatenate([w_k[h0], w_k[h1]], axis=1)
        wv_cat = np.concatenate([w_v[h0], w_v[h1]], axis=1)
        def lay8(w):  # [1024, 128] -> [128, DC, 128], prescaled into fp8
            return np.ascontiguousarray(
                (w * WSCALE).reshape(DC, 128, 128).transpose(1, 0, 2)
            ).astype(f8)
        def lay16(w):
            return np.ascontiguousarray(
                w.reshape(DC, 128, 128).transpose(1, 0, 2)).astype(bf)
        in_maps.append({
            "embedT": embedT,
            "embedT16": embedT16,
            "wq2": lay8(wq_cat),
            "wk2": lay8(wk_cat),
            "wv2": lay16(wv_cat),
            "wo2": np.ascontiguousarray(
                w_o[128 * c:128 * (c + 1), :]).astype(bf),
            "identin": ident,
        })
    return in_maps


def kernel(embed, pad_mask, w_q, w_k, w_v, w_o, _trace=False):
    from concourse.bass_utils import run_bass_kernel_spmd

    embed = np.asarray(embed, dtype=np.float32)
    w_q = np.asarray(w_q, dtype=np.float32)
    w_k = np.asarray(w_k, dtype=np.float32)
    w_v = np.asarray(w_v, dtype=np.float32)
    w_o = np.asarray(w_o, dtype=np.float32)

    if "nc" not in _cache:
        _cache["nc"] = _build_nc()
    nc = _cache["nc"]

    in_maps = _host_prep(embed, w_q, w_k, w_v, w_o)
    res = run_bass_kernel_spmd(nc, in_maps, core_ids=list(range(NCORES)),
                               trace=_trace)
    _cache["last_result"] = res
    out = np.zeros((B, S, D), dtype=np.float32)
    for r in res.results:
        out += r["outp"]
    return out


# revision 49
# speedup vs baseline: 1.0359x; 1.0359x over previous
# Multi-head causal attention (B=4, S=2048, D=1024, H=16, Dh=64) on 8 trn2 cores.
#
# Sharding: tensor-parallel over heads — core c owns heads (2c, 2c+1) for all
# batches. Each core projects Q/K/V for its 2 heads, runs causal attention, and
# computes a partial output projection against its 128 rows of w_o. The host
# sums the 8 partial outputs (the "all-reduce").
#
# Layouts (chosen so no transposes are needed on the attention path):
#   embedT   [B, 128, 8, S]  fp8e4  (q/k path; DoubleRow moving operand)
#   embedT16 [B, 128, 8, S]  bf16   (v path — fp8 V fails the accuracy gate:
#            h is a ~1000-key weighted average, so v's quantization noise
#            doesn't average down relative to h's small magnitude)
#   wq2/wk2  [128, 8, 128] fp8e4, prescaled x32 (clears e4m3 subnormals);
#            the 1/(32*32) comes back out inside the exp scale immediate
#   wv2      [128, 8, 128] bf16;  wo2 [128, 1024] bf16
#   Scores are computed transposed: sT[k, q] = sum_dh kT[dh,k] qT[dh,q]. The
#   Q/K projections run as fp8 DoubleRow matmuls (256-deep contraction, two
#   d-chunks per pass — half the PE passes of bf16). The softmax denominator
#   comes from a ones-column appended to V (the PV matmul accumulates both
#   the PV product and the exp-sum), and exp'd scores feed the PV matmul
#   directly as the moving operand.
#
# Engine assignment: PE is the bottleneck; causal masking runs as
# affine_select on gpsimd; ACT does exp (+ tail copies); DVE does PSUM evac;
# the normalize multiplies ride gpsimd (SBUF-only operands). Warm-up matmuls
# on garbage SBUF run during the initial DMA wait and inside phase_a's gaps
# so the PE's HAM clock gate reaches 8/8 (2.4 GHz) before the real stream.
#
# Softmax normalization: den row -> [128,8] reshape (SBUF->SBUF DMA) ->
# reciprocal -> DRAM -> [[0,64],[1,1024]] broadcast read. All stages after
# the reshape are DEFERRED (pend callbacks with chunk-count deadlines) so no
# queue head-blocks on an in-flight DMA. The final q-block of the final
# batch instead folds normalization into a per-head split of the output
# projection (row-tiled 64-contraction pairs) scaled by per-partition
# reciprocals — no broadcast DMA on the kernel tail's critical path.
import numpy as np
import ml_dtypes

B, S, D, H, Dh = 4, 2048, 1024, 16, 64
NCORES = 8
HPC = H // NCORES          # heads per core = 2
DC = D // 128              # d chunks = 8
NQB = S // 512             # q blocks = 4
NKB = S // 128             # k chunks = 16
NST = S // 128             # s tiles = 16
WSCALE = 32.0              # fp8 q/k weight prescale (clears e4m3 subnormals)
# 1/sqrt(Dh) folded into exp, divided by the q*k weight prescales
ESCALE = 1.0 / (8.0 * WSCALE * WSCALE)

_cache = {}


def _build_nc():
    import concourse.bass as bass
    import concourse.mybir as mybir
    import concourse.tile as tile
    from concourse import bacc

    bf16 = mybir.dt.bfloat16
    f32 = mybir.dt.float32
    f8 = mybir.dt.float8e4
    DRM = mybir.MatmulPerfMode.DoubleRow
    EXP = mybir.ActivationFunctionType.Exp
    GE = mybir.AluOpType.is_ge
    MUL = mybir.AluOpType.mult
    ADD = mybir.AluOpType.add

    nc = bacc.Bacc("TRN2", target_bir_lowering=False, debug=False,
                   num_devices=NCORES)

    embedT = nc.dram_tensor("embedT", [B, 128, DC, S], f8, kind="ExternalInput")
    embedT16 = nc.dram_tensor("embedT16", [B, 128, DC, S], bf16,
                              kind="ExternalInput")
    wq2 = nc.dram_tensor("wq2", [128, DC, 128], f8, kind="ExternalInput")
    wk2 = nc.dram_tensor("wk2", [128, DC, 128], f8, kind="ExternalInput")
    wv2 = nc.dram_tensor("wv2", [128, DC, 128], bf16, kind="ExternalInput")
    wo2 = nc.dram_tensor("wo2", [128, D], bf16, kind="ExternalInput")
    identin = nc.dram_tensor("identin", [128, 128], bf16, kind="ExternalInput")
    outp = nc.dram_tensor("outp", [B, S, D], bf16, kind="ExternalOutput")

    with tile.TileContext(nc) as tc:
        with (
            tc.tile_pool(name="const", bufs=1) as const,
            tc.tile_pool(name="etp", bufs=2) as etp,
            tc.tile_pool(name="etvp", bufs=2) as etvp,
            tc.tile_pool(name="qkp", bufs=2) as qkp,
            tc.tile_pool(name="vxp", bufs=2) as vxp,
            tc.tile_pool(name="hdp", bufs=2) as hdp,
            tc.tile_pool(name="expp", bufs=4) as expp,
            tc.tile_pool(name="denp", bufs=4) as denp,
            tc.tile_pool(name="outs", bufs=3) as outs,
            tc.tile_pool(name="tmpp", bufs=2) as tmpp,
            tc.tile_pool(name="pscore", bufs=2, space="PSUM") as pscore,
            tc.tile_pool(name="ppv", bufs=1, space="PSUM") as ppv,
            tc.tile_pool(name="pproj", bufs=2, space="PSUM") as pproj,
            tc.tile_pool(name="drp", bufs=8, space="DRAM") as drp,
        ):
            # HAM warm-up: tiny matmuls on zeroed SBUF, issued before
            # anything else so the PE is busy while the first weight and
            # embedding DMAs are in flight; by the time real matmuls start
            # the clock gate is at 8/8 instead of 4/8.
            junk = const.tile([128, 128], bf16, tag="junk")
            nc.vector.memset(junk[:], 0.0)
            wps = pscore.tile([128, 1024], f32, tag="score")
            for _ in range(64):
                nc.tensor.matmul(wps[:, 0:128], junk[:], junk[:])

            wq_sb = const.tile([128, DC, 128], f8, tag="wq")
            wk_sb = const.tile([128, DC, 128], f8, tag="wk")
            wv_sb = const.tile([128, DC, 128], bf16, tag="wv")
            wo_sb = const.tile([128, D], bf16, tag="wo")
            ident_sb = const.tile([128, 128], bf16, tag="ident")

            def load_et(b, prologue=False):
                """fp8 q/k-path chunks (gpsimd queue) + bf16 v-path chunks
                (sync queue; 4KB per-partition lines keep the DMA efficient).
                In the prologue the first fp8 dc pair and the weights lead
                both queues so phase_a starts as early as possible."""
                et = etp.tile([128, DC, S], f8, tag="et")
                etv = etvp.tile([128, DC, S], bf16, tag="etv")

                def chunk(dc):  # dc-pair: 4KB contiguous lines
                    nc.gpsimd.dma_start(out=et[:, dc:dc + 2, :],
                                        in_=embedT[b, :, dc:dc + 2, :])

                def chunkv(dc):  # dc-pair: 8KB contiguous lines
                    nc.sync.dma_start(out=etv[:, dc:dc + 2, :],
                                      in_=embedT16[b, :, dc:dc + 2, :])
                if prologue:
                    chunk(0)
                    nc.gpsimd.dma_start(out=wq_sb[:], in_=wq2[:])
                    nc.gpsimd.dma_start(out=wk_sb[:], in_=wk2[:])
                    nc.sync.dma_start(out=wv_sb[:], in_=wv2[:])
                    chunkv(0)
                    nc.sync.dma_start(out=ident_sb[:], in_=identin[:])
                    for dc in range(2, DC, 2):
                        chunk(dc)
                    nc.sync.dma_start(out=wo_sb[:], in_=wo2[:])
                    for dc in range(2, DC, 2):
                        chunkv(dc)
                else:
                    for dc in range(0, DC, 2):
                        chunk(dc)
                    for dc in range(0, DC, 2):
                        chunkv(dc)
                return et, etv

            def make_proj_units(ets, b, prologue=False):
                """Units producing qT2, kT2, vext0/1 for one batch. Q/K are
                fp8 DoubleRow from `et`; V is bf16 from `etv`."""
                et, etv = ets
                qT2 = qkp.tile([128, S], bf16, tag="qT2")
                kT2 = qkp.tile([128, S], bf16, tag="kT2")
                vT2 = qkp.tile([128, S], bf16, tag="vT2")
                vext0 = vxp.tile([128, NKB, 65], bf16, tag="vext0")
                vext1 = vxp.tile([128, NKB, 65], bf16, tag="vext1")
                units = []

                def ones_u():
                    nc.gpsimd.memset(vext0[:, :, 64:65], 1.0)
                    nc.gpsimd.memset(vext1[:, :, 64:65], 1.0)
                units.append(ones_u)
                if prologue:
                    # dc-major over the first q/k blocks so the PE starts as
                    # soon as the first embedding chunk pair lands; warm-up
                    # matmuls fill the DMA-wait gaps
                    def phase_a():
                        ps_q = pproj.tile([128, 512], f32, tag="proj")
                        ps_k = pproj.tile([128, 512], f32, tag="proj")
                        for dc in range(0, DC, 2):
                            nc.tensor.matmul(ps_q[:], wq_sb[:, dc:dc + 2, :],
                                             et[:, dc:dc + 2, 0:512],
                                             start=(dc == 0),
                                             stop=(dc == DC - 2),
                                             perf_mode=DRM)
                            nc.tensor.matmul(ps_k[:], wk_sb[:, dc:dc + 2, :],
                                             et[:, dc:dc + 2, 0:512],
                                             start=(dc == 0),
                                             stop=(dc == DC - 2),
                                             perf_mode=DRM)
                            for _ in range(4):
                                nc.tensor.matmul(wps[:, 0:128], junk[:],
                                                 junk[:])
                        nc.vector.tensor_copy(out=qT2[:, 0:512], in_=ps_q[:])
                        nc.vector.tensor_copy(out=kT2[:, 0:512], in_=ps_k[:])
                    units.append(phase_a)
                for dst, w_sb in ((qT2, wq_sb), (kT2, wk_sb)):
                    for sblk in range(S // 512):
                        if prologue and sblk == 0:
                            continue
                        def proj_u(dst=dst, w_sb=w_sb, sblk=sblk):
                            ps = pproj.tile([128, 512], f32, tag="proj")
                            for dc in range(0, DC, 2):
                                nc.tensor.matmul(
                                    ps[:], w_sb[:, dc:dc + 2, :],
                                    et[:, dc:dc + 2,
                                        sblk * 512:(sblk + 1) * 512],
                                    start=(dc == 0), stop=(dc == DC - 2),
                                    perf_mode=DRM)
                            nc.vector.tensor_copy(
                                out=dst[:, sblk * 512:(sblk + 1) * 512],
                                in_=ps[:])
                        units.append(proj_u)
                # v path: bf16 from etv
                for sblk in range(S // 512):
                    def vproj_u(sblk=sblk):
                        ps = pproj.tile([128, 512], f32, tag="proj")
                        for dc in range(DC):
                            nc.tensor.matmul(
                                ps[:], wv_sb[:, dc, :],
                                etv[:, dc, sblk * 512:(sblk + 1) * 512],
                                start=(dc == 0), stop=(dc == DC - 1))
                        nc.vector.tensor_copy(
                            out=vT2[:, sblk * 512:(sblk + 1) * 512], in_=ps[:])
                    units.append(vproj_u)
                for st in range(NST):
                    def tr_u(st=st):
                        vt = pproj.tile([128, 128], bf16, tag="proj")
                        nc.tensor.transpose(
                            vt[:], vT2[:, st * 128:(st + 1) * 128], ident_sb[:])
                        nc.vector.tensor_copy(out=vext0[:, st, 0:64],
                                              in_=vt[:, 0:64])
                        nc.vector.tensor_copy(out=vext1[:, st, 0:64],
                                              in_=vt[:, 64:128])
                    units.append(tr_u)
                return units, (qT2, kT2, vext0, vext1)

            def make_outproj_units(bb, hq, sts=range(NST), act_half=False):
                """Outproj units; consecutive (even,odd) s-tiles share one
                [128, 2, 1024] output tile and a single paired 512KB store,
                halving DMA-issue count. Stores alternate gpsimd/sync."""
                units = []
                state = {}
                sts = list(sts)
                for idx, st in enumerate(sts):
                    def op_u(st=st, idx=idx):
                        hs = hq[st // 4][:, (st % 4) * 128:(st % 4 + 1) * 128]
                        po0 = pproj.tile([128, 512], f32, tag="proj")
                        nc.tensor.matmul(po0[:], hs, wo_sb[:, 0:512])
                        po1 = pproj.tile([128, 512], f32, tag="proj")
                        nc.tensor.matmul(po1[:], hs, wo_sb[:, 512:1024])
                        if idx % 2 == 0:
                            state["ob2"] = outs.tile([128, 2, 1024], bf16,
                                                     tag="ob2", name="ob2")
                        ob = state["ob2"][:, idx % 2, :]
                        if act_half:
                            # splitting one evac to the (then-idle) ACT keeps
                            # the PSUM slots recycling at PE pace
                            nc.scalar.copy(out=ob[:, 0:512], in_=po0[:])
                        else:
                            nc.vector.tensor_copy(out=ob[:, 0:512], in_=po0[:])
                        nc.vector.tensor_copy(out=ob[:, 512:1024], in_=po1[:])
                        if idx % 2 == 1:
                            emit_store(bb, st - 1, state["ob2"])
                    units.append(op_u)
                return units

            def emit_store(bb, st0, ob2):
                # paired store: rows [st0*128, st0*128+256) of outp
                d0 = outp[bb, st0 * 128:(st0 + 1) * 128, :]
                dst = bass.AP(tensor=d0.tensor, offset=d0.offset,
                              ap=[[D, 128], [128 * D, 2], [1, D]])
                q = nc.gpsimd if (st0 // 2) % 2 == 0 else nc.sync
                q.dma_start(out=dst, in_=ob2[:])

            def tail_outproj_units(bb, hu, denct):
                """Final q-block of the final batch: h is UNnormalized; the
                output projection is split per head (row-tiled 64-contraction
                pairs run concurrently on the PE) and the per-q reciprocal
                denominators are applied as per-partition scalars during the
                PSUM combine — no broadcast DMA on the tail critical path."""
                units = []
                state = {}
                for idx, st in enumerate(range(12, 16)):
                    def op_u(st=st, idx=idx):
                        sub = st % 4
                        hs0 = hu[0:64, sub * 128:(sub + 1) * 128]
                        hs1 = hu[64:128, sub * 128:(sub + 1) * 128]
                        r0 = denct[:, 8 + sub:9 + sub]
                        r1 = denct[:, 12 + sub:13 + sub]
                        ph0 = pscore.tile([128, 1024], f32, tag="score")
                        ph1 = pscore.tile([128, 1024], f32, tag="score")
                        nc.tensor.matmul(ph0[:, 0:512], hs0, wo_sb[0:64, 0:512])
                        nc.tensor.matmul(ph1[:, 0:512], hs1,
                                         wo_sb[64:128, 0:512])
                        nc.tensor.matmul(ph0[:, 512:1024], hs0,
                                         wo_sb[0:64, 512:1024])
                        nc.tensor.matmul(ph1[:, 512:1024], hs1,
                                         wo_sb[64:128, 512:1024])
                        if idx % 2 == 0:
                            state["ob2"] = outs.tile([128, 2, 1024], bf16,
                                                     tag="ob2", name="ob2")
                        ob = state["ob2"][:, idx % 2, :]
                        tmp0 = tmpp.tile([128, 512], f32, tag="tmp0",
                                         name="tmp0")
                        tmp1 = tmpp.tile([128, 512], f32, tag="tmp1",
                                         name="tmp1")
                        # ob = po_h0 * (1/den_h0[q]) + po_h1 * (1/den_h1[q]),
                        # den indexed per PARTITION (q) in this layout; the
                        # first product rides the (tail-idle) ACT so the DVE
                        # only runs the fused multiply-adds
                        nc.scalar.activation(
                            out=tmp0[:], in_=ph0[:, 0:512],
                            func=mybir.ActivationFunctionType.Copy, scale=r0)
                        nc.vector.scalar_tensor_tensor(
                            ob[:, 0:512], ph1[:, 0:512], r1, tmp0[:],
                            op0=MUL, op1=ADD)
                        nc.scalar.activation(
                            out=tmp1[:], in_=ph0[:, 512:1024],
                            func=mybir.ActivationFunctionType.Copy, scale=r0)
                        nc.vector.scalar_tensor_tensor(
                            ob[:, 512:1024], ph1[:, 512:1024], r1, tmp1[:],
                            op0=MUL, op1=ADD)
                        if idx % 2 == 1:
                            emit_store(bb, st - 1, state["ob2"])
                    units.append(op_u)
                return units

            def run_attention(b, proj_tiles, units, hq, pend, gct, last=False,
                              post=None):
                """Attention for batch b; `units` paced into the kb loop.
                `pend` carries deferred (deadline, callback) stages — the
                reciprocal, its DMAs, and the normalize multiplies — so no
                engine stream ever waits on an in-flight DMA. `gct` is a
                global chunk counter (survives across batches)."""
                qT2, kT2, vext0, vext1 = proj_tiles
                nslots = sum(4 * qb + 4 for qb in range(NQB))
                t = 0
                tail = {}
                for qb in range(NQB):
                    qs = slice(qb * 512, (qb + 1) * 512)
                    pv0 = ppv.tile([65, 512], f32, tag="pv0")
                    pv1 = ppv.tile([65, 512], f32, tag="pv1")
                    nkb = 4 * qb + 4
                    exs = [None] * nkb

                    def emit_qk(kb):
                        ps = pscore.tile([128, 1024], f32, tag="score")
                        ks = slice(kb * 128, (kb + 1) * 128)
                        diag = kb >= 4 * qb
                        # columns left of the diagonal triangle are fully
                        # masked — skip them in the QK stream too
                        w0 = (kb - 4 * qb) * 128 if diag else 0
                        qsw = slice(qb * 512 + w0, (qb + 1) * 512)
                        nc.tensor.matmul(ps[:, w0:512], kT2[0:64, ks],
                                         qT2[0:64, qsw])
                        nc.tensor.matmul(ps[:, 512 + w0:1024], kT2[64:128, ks],
                                         qT2[64:128, qsw])
                        ex = expp.tile([128, 1024], bf16, tag="ex")
                        if diag:
                            # exp covers [w0:512]; the 128-wide triangle block
                            # is then zeroed below the diagonal by an
                            # affine_select on the gpsimd engine (iota =
                            # q_local - k_local, keep iff >= 0)
                            ex3 = ex.rearrange("p (h n) -> p h n", h=2)
                            ps3 = ps.rearrange("p (h n) -> p h n", h=2)
                            nc.scalar.activation(out=ex3[:, :, w0:512],
                                                 in_=ps3[:, :, w0:512],
                                                 func=EXP, scale=ESCALE)
                            for h in (0, 1):
                                nc.gpsimd.affine_select(
                                    out=ex3[:, h, w0:w0 + 128],
                                    in_=ex3[:, h, w0:w0 + 128],
                                    pattern=[[1, 128]],
                                    compare_op=GE, fill=0.0,
                                    channel_multiplier=-1)
                        else:
                            nc.scalar.activation(out=ex[:], in_=ps[:],
                                                 func=EXP, scale=ESCALE)
                        exs[kb] = ex

                    def emit_pv(kb):
                        # diagonal chunks contribute nothing to q-columns left
                        # of the triangle block; skip them (ex is garbage
                        # there — it was never exp'd).
                        first, last_ = (kb == 0), (kb == nkb - 1)
                        w0 = (kb - 4 * qb) * 128 if kb >= 4 * qb else 0
                        ex = exs[kb]
                        nc.tensor.matmul(pv0[:, w0:512],
                                         vext0[:, kb, :],
                                         ex[:, w0:512],
                                         start=first, stop=last_)
                        nc.tensor.matmul(pv1[:, w0:512],
                                         vext1[:, kb, :],
                                         ex[:, 512 + w0:1024],
                                         start=first, stop=last_)

                    for kb in range(nkb):
                        emit_qk(kb)
                        t += 1
                        gct[0] += 1
                        # flush deferred den-chain stages whose deadlines
                        # passed; deadlines give each stage's input DMA ~3
                        # chunks to land so nothing head-blocks a queue
                        while pend and gct[0] >= pend[0][0]:
                            pend.pop(0)[1]()
                        if units:
                            u = units.pop(0)
                            if u is not None:
                                u()
                        while units and len(units) > nslots - t:
                            u = units.pop(0)
                            if u is not None:
                                u()
                        if kb > 1:
                            emit_pv(kb - 2)
                    emit_pv(nkb - 2)
                    emit_pv(nkb - 1)

                    if last and qb == NQB - 1:
                        # tail path: unnormalized h straight from PSUM (bf16),
                        # and the den row reshaped so that denct[p, 4h+sub]
                        # = den_h[sub*128 + p] — per-PARTITION reciprocals
                        # for the per-head split output projection.
                        hu = hq[qb]
                        # den rows stage through SBUF on the ACT (runs in
                        # parallel with the DVE h copies); DMA cannot read
                        # PSUM directly
                        denrow = denp.tile([1, 1024], f32, tag="denrow",
                                           name="denrow", bufs=1)
                        nc.scalar.copy(out=denrow[0:1, 0:512],
                                       in_=pv0[64:65, :])
                        nc.scalar.copy(out=denrow[0:1, 512:1024],
                                       in_=pv1[64:65, :])
                        nc.vector.tensor_copy(out=hu[0:64, :],
                                              in_=pv0[0:64, :])
                        nc.vector.tensor_copy(out=hu[64:128, :],
                                              in_=pv1[0:64, :])
                        denct = denp.tile([128, 16], f32, tag="denc")
                        # transposed reshape via a DRAM bounce (the SBUF->SBUF
                        # balancer can't split a partition-transposing AP):
                        # denct[p, 4h+j] = den_h[j*128 + p]
                        ddenr = drp.tile([1, 1024], f32, tag="dden2")
                        nc.sync.dma_start(out=ddenr[:], in_=denrow[:])
                        gap = bass.AP(tensor=ddenr.tensor, offset=ddenr.offset,
                                      ap=[[1, 128], [128, 8]])
                        nc.sync.dma_start(out=denct[:, 0:8], in_=gap)
                        nc.vector.reciprocal(out=denct[:, 8:16],
                                             in_=denct[:, 0:8])
                        tail["denct"] = denct
                        continue

                    # normalize, in deferred stages:
                    #   A (now):      pvs copy to SBUF (frees the PSUM slot)
                    #                 + den-row reshape to [128,8] via
                    #                 SBUF->SBUF DMA
                    #   B (+3 chunks): reciprocal [128,8] (its DMA input has
                    #                 landed, so the DVE never stalls), then
                    #                 recip -> DRAM -> [[0,64],[1,1024]]
                    #                 broadcast read (partition broadcast)
                    #   C (+7 chunks): normalize multiplies (gpsimd — all
                    #                 SBUF operands; keeps the DVE free for
                    #                 PSUM evacuation)
                    pvs2 = denp.tile([65, 1024], f32, tag="pvs")
                    nc.vector.tensor_copy(out=pvs2[:, 0:512], in_=pv0[:])
                    nc.vector.tensor_copy(out=pvs2[:, 512:1024], in_=pv1[:])
                    denc = denp.tile([128, 16], f32, tag="denc")
                    # SBUF->SBUF reshape: dst iterates partition-major so
                    # denc[p, j] = den_row[8p + j] in one DMA (p<64: head0)
                    nc.sync.dma_start(out=denc[:, 0:8], in_=pvs2[64:65, :])
                    dden2 = drp.tile([1, 1024], f32, tag="dden2")
                    den2 = denp.tile([64, 1024], f32, tag="den")

                    def stage_b(denc=denc, dden2=dden2, den2=den2):
                        nc.vector.reciprocal(out=denc[:, 8:16],
                                             in_=denc[:, 0:8])
                        nc.sync.dma_start(out=dden2[:], in_=denc[:, 8:16])
                        bcap = bass.AP(tensor=dden2.tensor,
                                       offset=dden2.offset,
                                       ap=[[0, 64], [1, 1024]])
                        nc.sync.dma_start(out=den2[:], in_=bcap)

                    def stage_c(pvs2=pvs2, den2=den2, ht2=hq[qb]):
                        for h in (0, 1):
                            nc.vector.tensor_mul(
                                ht2[h * 64:(h + 1) * 64, :],
                                pvs2[0:64, h * 512:(h + 1) * 512],
                                den2[:, h * 512:(h + 1) * 512])

                    db, dcm = (3, 5) if last else (3, 7)
                    pend.append((gct[0] + db, stage_b))
                    pend.append((gct[0] + dcm, stage_c))
                    # outproj units consuming this qb's h MUST be emitted
                    # after stage_c: the tile framework links a read emitted
                    # before its writer to the slot's PREVIOUS write (stale
                    # data), so emission order is a correctness requirement
                    if post and qb in post:
                        for u in post[qb]:
                            pend.append((gct[0] + dcm + 1, u))
                while units:
                    u = units.pop(0)
                    if u is not None:
                        u()
                return tail

            # prologue: batch 0 projections run standalone
            et0 = load_et(0, prologue=True)
            units0, tiles0 = make_proj_units(et0, 0, prologue=True)
            for u in units0:
                u()

            cur_tiles = tiles0
            prev_hq = None
            pend = []
            gct = [0]
            for b in range(B):
                hq = [hdp.tile([128, 512], bf16, tag=f"h{i}", name=f"hq{i}")
                      for i in range(NQB)]
                units = []
                if b > 0:
                    # st12-15 of the previous batch read hq[3] whose muls
                    # only flush a few chunks into THIS batch — pend-gate
                    # them behind that flush (emission order = correctness)
                    units += make_outproj_units(b - 1, prev_hq, range(0, 12))
                    gdl = pend[-1][0] + 1 if pend else 0
                    for u in make_outproj_units(b - 1, prev_hq,
                                                range(12, 16)):
                        pend.append((gdl, u))
                post = None
                if b + 1 < B:
                    et_n = load_et(b + 1)
                    punits, next_tiles = make_proj_units(et_n, b + 1)
                    # interleave: outproj units first (their deps are ready
                    # while et(b+1) is still streaming in), then alternate
                    k = min(8, len(units))
                    head, rest = units[:k], units[k:]
                    mixed = []
                    i = j = 0
                    while i < len(rest) or j < len(punits):
                        if j < len(punits):
                            mixed.append(punits[j]); j += 1
                        if i < len(rest):
                            mixed.append(rest[i]); i += 1
                    units = head + mixed
                else:
                    next_tiles = None
                    # last batch: its own outproj units are pend-gated so
                    # each group is emitted only after its qb's normalize
                    # muls (see run_attention)
                    post = {
                        0: make_outproj_units(b, hq, range(0, 4)),
                        1: make_outproj_units(b, hq, range(4, 8)),
                        2: make_outproj_units(b, hq, range(8, 12),
                                              act_half=True),
                    }
                tail = run_attention(b, cur_tiles, units, hq, pend, gct,
                                     last=(b == B - 1), post=post)
                cur_tiles = next_tiles
                prev_hq = hq

            for _, fn in pend:
                fn()
            for u in tail_outproj_units(B - 1, prev_hq[NQB - 1],
                                        tail["denct"]):
                u()

    nc.compile()
    return nc


def _host_prep(embed, w_q, w_k, w_v, w_o):
    bf = ml_dtypes.bfloat16
    f8 = ml_dtypes.float8_e4m3  # bit-compatible with TRN fp8e4 for |x|<=240
    embedT16 = np.ascontiguousarray(
        embed.reshape(B, S, DC, 128).transpose(0, 3, 2, 1)).astype(bf)
    embedT = embedT16.astype(f8)
    ident = np.ascontiguousarray(np.eye(128, dtype=np.float32).astype(bf))

    in_maps = []
    for c in range(NCORES):
        h0, h1 = HPC * c, HPC * c + 1
        wq_cat = np.concatenate([w_q[h0], w_q[h1]], axis=1)
        wk_cat = np.concatenate([w_k[h0], w_k[h1]], axis=1)
        wv_cat = np.concatenate([w_v[h0], w_v[h1]], axis=1)
        def lay8(w):  # [1024, 128] -> [128, DC, 128], prescaled into fp8
            return np.ascontiguousarray(
                (w * WSCALE).reshape(DC, 128, 128).transpose(1, 0, 2)
            ).astype(f8)
        def lay16(w):
            return np.ascontiguousarray(
                w.reshape(DC, 128, 128).transpose(1, 0, 2)).astype(bf)
        in_maps.append({
            "embedT": embedT,
            "embedT16": embedT16,
            "wq2": lay8(wq_cat),
            "wk2": lay8(wk_cat),
            "wv2": lay16(wv_cat),
            "wo2": np.ascontiguousarray(
                w_o[128 * c:128 * (c + 1), :]).astype(bf),
            "identin": ident,
        })
    return in_maps


def kernel(embed, pad_mask, w_q, w_k, w_v, w_o, _trace=False):
    from concourse.bass_utils import run_bass_kernel_spmd

    embed = np.asarray(embed, dtype=np.float32)
    w_q = np.asarray(w_q, dtype=np.float32)
    w_k = np.asarray(w_k, dtype=np.float32)
    w_v = np.asarray(w_v, dtype=np.float32)
    w_o = np.asarray(w_o, dtype=np.float32)

    if "nc" not in _cache:
        _cache["nc"] = _build_nc()
    nc = _cache["nc"]

    in_maps = _host_prep(embed, w_q, w_k, w_v, w_o)
    res = run_bass_kernel_spmd(nc, in_maps, core_ids=list(range(NCORES)),
                               trace=_trace)
    _cache["last_result"] = res
    out = np.zeros((B, S, D), dtype=np.float32)
    for r in res.results:
        out += r["outp"]
    return out
